# revision 31
# baseline (speedup 1.0000x reference)
"""Trainium2 Bass kernel for nn_HardLinearAttention.

Math: out = Z + (alpha/n) * P @ Z @ M @ Z.T @ Q @ Z with
  P = e_last e_last^T, M = lower-tri lambda^(i-j) (last row/col zero),
  Q = [[-I, I],[0,0]] blocks.
Because P has a single nonzero (bottom-right), the update is rank-1: only the
last row of the output differs from Z.  With z = Z[-1,:] (masked at col n):
  r[j] = sum_k lambda^k z[j+k]          (geometric window, 64 taps:
                                         lambda^64 ~ 1.2e-3, far below the
                                         bf16 quantization already accepted)
  s[i] = sum_j Z[i,j] r[j]   (i < d)    (only s[0:d] survives Q)
  u[j] = sum_k s[k] (Z[d+k,j] - Z[k,j])
  out[-1,:] = Z[-1,:] + (alpha/n) u ;  out[i,:] = Z[i,:] otherwise.

Sharding: context axis (n+1) split 8 ways (1025 cols/core over a zero-padded
8200-wide array).  Each core computes r already broadcast across partitions
in one matmul per chunk (lamB[k,p] = lambda^k as the weight against the
shifted-window toeplitz), forms its partial s columns with multiply+reduce
against the bulk-loaded Ztop tiles, a 2KB DRAM AllGather + local sum
combines s across cores, then each core computes u for its columns.

Performance structure (final):
  - Bulk copy moves as bf16 (~1.7e-3 rel err, under the 2e-2 gate), halving
    DMA traffic.  The updated last row stays f32.
  - No transposed Ztop input: stage 2 is elementwise multiply+reduce against
    zbig rows 0..511, which are loaded first (tile-major order) with big
    descriptors.  This removes the 256-small-descriptor ztp load whose
    cross-core ring skew previously delayed the collective by ~30us.
  - Queues: SP carries the tiny loads, then the rows-0..511 half of the bulk
    load, then the last-row store; Act carries the rows-512..1023 half plus
    ALL bulk stores; the Pool/SWDGE queue carries ONLY the collective's
    DMAs -- pending SWDGE descriptors delay the CC mesh start, so keeping
    that queue empty is worth ~20us.
"""

import sys

for _p in ("/opt/trn_rl_repo", "/root/.axon_site/_ro/trn_rl_repo"):
    if _p not in sys.path:
        sys.path.append(_p)

import ml_dtypes
import numpy as np

import concourse.bacc as bacc
import concourse.bass as bass
import concourse.mybir as mybir
import concourse.tile as tile
from concourse.ap import AP
from concourse import bass_utils

F32 = mybir.dt.float32
BF16 = mybir.dt.bfloat16
NP_BF16 = ml_dtypes.bfloat16

D = 512          # feature dim d
N = 8192         # context length n
R = 2 * D + 1    # 1025 rows
NC = 8           # cores
L = 1025         # columns per core (8 * 1025 = 8200 >= 8193)
WTOT = NC * L    # 8200 padded width
W = 64           # geometric window taps
LPAD = 1152      # padded local column count (3 chunks of 384)
ZWLEN = 1280     # zwin input length: covers LPAD + W - 1
NT_K = D // 128      # 4 feature tiles
NT_ROW = 8           # full 128-row tiles (rows 0..1023)
RT_CHUNK = 384       # rT is produced in 3 chunks of 384 columns
J_CHUNKS = [(0, 512), (512, 1024), (1024, 1025)]

_PROGRAM = None


def _build_program():
    nc = bacc.Bacc(
        "TRN2",
        target_bir_lowering=False,
        debug=False,
        enable_asserts=False,
        num_devices=NC,
    )

    zc_d = nc.dram_tensor("zc", [128, NT_ROW, L], BF16, kind="ExternalInput")
    zlast_d = nc.dram_tensor("zlast", [L], F32, kind="ExternalInput")
    zwin_d = nc.dram_tensor("zwin", [ZWLEN], BF16, kind="ExternalInput")
    lamb_d = nc.dram_tensor("lamb", [W, 128], BF16, kind="ExternalInput")
    alpha_d = nc.dram_tensor("alpha", [1], F32, kind="ExternalInput")
    out_d = nc.dram_tensor("out", [128, NT_ROW, L], BF16, kind="ExternalOutput")
    outlast_d = nc.dram_tensor("outlast", [L], F32, kind="ExternalOutput")

    with tile.TileContext(nc) as tc:
        with (
            tc.tile_pool(name="consts", bufs=1) as consts,
            tc.tile_pool(name="zbuf", bufs=1) as zbuf,
            tc.tile_pool(name="work", bufs=1) as work,
            tc.tile_pool(name="rt_ps", bufs=2, space=bass.MemorySpace.PSUM) as rt_ps,
            tc.tile_pool(name="rb_ps", bufs=2, space=bass.MemorySpace.PSUM) as rb_ps,
            tc.tile_pool(name="u_ps", bufs=2, space=bass.MemorySpace.PSUM) as u_ps,
            tc.tile_pool(name="ccdram", bufs=1, space="DRAM") as ccdram,
        ):
            # ---- SP queue: critical small loads + the 0..511-row half of
            # the bulk load (partitions 0-63) ------------------------------
            lamB = consts.tile([W, 128], BF16, name="lamB")
            nc.sync.dma_start(lamB[:], lamb_d[:, :])

            # overlapping window: win[k, j] = zwin[k + j]
            win = consts.tile([W, LPAD], BF16, name="win")
            nc.sync.dma_start(win[:], AP(zwin_d, 0, [[1, W], [1, LPAD]]))

            zbig = zbuf.tile([128, NT_ROW, L], BF16, name="zbig")
            nc.sync.dma_start(zbig[0:64, 0:4, :], zc_d[0:64, 0:4, :])

            zlast = work.tile([1, L], F32, name="zlast")
            nc.sync.dma_start(zlast[:], zlast_d[:].unsqueeze(0))
            alpha_sb = consts.tile([1, 1], F32, name="alpha_sb")
            nc.sync.dma_start(alpha_sb[:], alpha_d[0:1].unsqueeze(1))

            scale_sb = consts.tile([1, 1], F32, name="scale_sb")
            nc.vector.tensor_scalar_mul(scale_sb[:], alpha_sb[:], 1.0 / float(N))

            # ---- Act queue: the other 0..511-row half first --------------
            nc.scalar.dma_start(zbig[64:128, 0:4, :], zc_d[64:128, 0:4, :])

            # ---- stage 1: rbc = lamB.T @ win directly gives the r row
            # broadcast across all 128 partitions (lamB[k, p] = lambda^k) --
            rbc = work.tile([128, LPAD], BF16, name="rbc")
            for c in range(3):
                c0, c1 = c * RT_CHUNK, (c + 1) * RT_CHUNK
                rb = rb_ps.tile([128, RT_CHUNK], F32, name="rb", tag="rb")
                nc.tensor.matmul(rb[:], lamB[:], win[:, c0:c1],
                                 start=True, stop=True)
                nc.vector.tensor_copy(rbc[:, c0:c1], rb[:])

            # ---- stage 2: fused multiply+reduce against Ztop tiles ------
            # s[i] = sum_j zbig[i, j] * r[j] for the 4 feature tiles
            s_sb = work.tile([128, NT_K], F32, name="s_sb")
            for kt in range(NT_K):
                prod = work.tile([128, L], BF16, name=f"prod{kt}", tag=f"prod{kt}")
                nc.vector.tensor_mul(prod[:], zbig[:, kt, :], rbc[:, 0:L])
                nc.vector.tensor_reduce(
                    s_sb[:, kt:kt + 1], prod[:],
                    mybir.AxisListType.X, mybir.AluOpType.add,
                )

            # delay B's descriptor GENERATION until the local A half has
            # landed: rings serve descriptors in arrival order, so B must
            # not be enqueued before every core's A descriptors are in
            gate_dr = ccdram.tile([2, 2], BF16, name="gate_dr")
            nc.scalar.dma_start(gate_dr[0:1, :], zbig[63:64, 3, L - 2:L])
            nc.scalar.dma_start(gate_dr[1:2, :], zbig[127:128, 3, L - 2:L])
            nc.scalar.dma_start(zbig[0:64, 4:8, :], zc_d[0:64, 4:8, :])
            nc.scalar.dma_start(zbig[64:128, 4:8, :], zc_d[64:128, 4:8, :])

            # ---- AllGather partial s (2 KB) + local sum -----------------
            cc_in = ccdram.tile([128, NT_K], F32, name="cc_in")
            cc_out = ccdram.tile([NC * 128, NT_K], F32, name="cc_out")
            nc.gpsimd.dma_start(cc_in[:], s_sb[:])
            nc.gpsimd.collective_compute(
                "AllGather",
                mybir.AluOpType.bypass,
                replica_groups=[list(range(NC))],
                ins=[cc_in.opt()],
                outs=[cc_out.opt()],
            )
            sg = work.tile([128, NC, NT_K], F32, name="sg")
            nc.gpsimd.dma_start(sg[:], cc_out.rearrange("(r p) c -> p r c", p=128))

            # ---- bulk stores, all on the Act queue (keeping the Pool
            # queue empty: the collective's own DMA advances FIFO behind
            # pending SWDGE descriptors, so stores there delay the mesh) --
            for q in range(4):
                nc.scalar.dma_start(
                    out_d[q * 32:(q + 1) * 32, :, :],
                    zbig[q * 32:(q + 1) * 32, :, :],
                )

            # ---- stage 3 prep: zd = Zmid - Ztop (emitted before the
            # post-collective vector work so the in-order vector engine
            # isn't stalled on the mesh) ----------------------------------
            zd = []
            for kt in range(NT_K):
                zd_t = work.tile([128, L], BF16, name=f"zd{kt}", tag=f"zd{kt}")
                nc.vector.tensor_sub(zd_t[:], zbig[:, NT_K + kt, :], zbig[:, kt, :])
                zd.append(zd_t)

            # ---- post-collective: sum the 8 partial s, cast to bf16 -----
            ssum = work.tile([128, NT_K], F32, name="ssum")
            nc.vector.tensor_add(ssum[:], sg[:, 0, :], sg[:, 1, :])
            for r_ in range(2, NC):
                nc.vector.tensor_add(ssum[:], ssum[:], sg[:, r_, :])
            ssum_bf = work.tile([128, NT_K], BF16, name="ssum_bf")
            nc.vector.tensor_copy(ssum_bf[:], ssum[:])

            # ---- stage 3: u = zd.T @ s; last row = zlast + scale*u ------
            newrow = work.tile([1, L], F32, name="newrow")
            for (j0, j1) in J_CHUNKS:
                u = u_ps.tile([1, j1 - j0], F32, name="u", tag="u")
                for kt in range(NT_K):
                    nc.tensor.matmul(
                        u[:], ssum_bf[:, kt:kt + 1], zd[kt][:, j0:j1],
                        start=(kt == 0), stop=(kt == NT_K - 1),
                    )
                nc.vector.scalar_tensor_tensor(
                    newrow[:, j0:j1], u[:], scale_sb[:], zlast[:, j0:j1],
                    op0=mybir.AluOpType.mult, op1=mybir.AluOpType.add,
                )
            nc.sync.dma_start(outlast_d[:].unsqueeze(0), newrow[:])

    nc.compile()
    return nc


def _get_program():
    global _PROGRAM
    if _PROGRAM is None:
        _PROGRAM = _build_program()
    return _PROGRAM


def _make_in_maps(Z, alpha, M=None):
    Z = np.asarray(Z, dtype=np.float32)
    alpha = np.asarray(alpha, dtype=np.float32).reshape(1)
    # lambda powers; prefer deriving from M's first column when provided.
    if M is not None:
        lam = np.ascontiguousarray(np.asarray(M)[0:W, 0], dtype=np.float32)
    else:
        lam = (0.9 ** np.arange(W)).astype(np.float32)
    lamb_bf = np.ascontiguousarray(
        np.broadcast_to(lam[:, None], (W, 128))
    ).astype(NP_BF16)

    Zp = np.zeros((R, WTOT), dtype=np.float32)
    Zp[:, : N + 1] = Z
    zmpad = np.zeros(WTOT + ZWLEN, dtype=np.float32)
    zmpad[:N] = Z[R - 1, :N]  # col n masked to zero (M's last row is zero)

    in_maps = []
    for c in range(NC):
        j0 = c * L
        shard = Zp[:, j0:j0 + L]
        # rows 0..1023 permuted: zc[p, t, :] = shard[t*128 + p, :], bf16
        zc = np.ascontiguousarray(
            shard[:1024].reshape(NT_ROW, 128, L).transpose(1, 0, 2)
        ).astype(NP_BF16)
        in_maps.append(
            {
                "zc": zc,
                "zlast": np.ascontiguousarray(shard[R - 1]),
                "zwin": np.ascontiguousarray(zmpad[j0:j0 + ZWLEN]).astype(NP_BF16),
                "lamb": lamb_bf,
                "alpha": alpha,
            }
        )
    return in_maps


def kernel(Z, alpha, P=None, M=None, Q=None, **_ignored):
    nc = _get_program()
    in_maps = _make_in_maps(Z, alpha, M)
    res = bass_utils.run_bass_kernel_spmd(nc, in_maps, core_ids=list(range(NC)))
    full = np.zeros((R, WTOT), dtype=np.float32)
    for c in range(NC):
        j0 = c * L
        rows = (
            res.results[c]["out"].astype(np.float32)
            .transpose(1, 0, 2).reshape(1024, L)
        )
        full[:1024, j0:j0 + L] = rows
        full[R - 1, j0:j0 + L] = res.results[c]["outlast"]
    return full[:, : N + 1].astype(np.float32)


# revision 32
# speedup vs baseline: 1.0940x; 1.0940x over previous
"""Trainium2 Bass kernel for nn_HardLinearAttention.

Math: out = Z + (alpha/n) * P @ Z @ M @ Z.T @ Q @ Z with
  P = e_last e_last^T, M = lower-tri lambda^(i-j) (last row/col zero),
  Q = [[-I, I],[0,0]] blocks.
Because P has a single nonzero (bottom-right), the update is rank-1: only the
last row of the output differs from Z.  With z = Z[-1,:] (masked at col n):
  r[j] = sum_k lambda^k z[j+k]          (geometric window, 64 taps:
                                         lambda^64 ~ 1.2e-3, far below the
                                         bf16 quantization already accepted)
  s[i] = sum_j Z[i,j] r[j]   (i < d)    (only s[0:d] survives Q)
  u[j] = sum_k s[k] (Z[d+k,j] - Z[k,j])
  out[-1,:] = Z[-1,:] + (alpha/n) u ;  out[i,:] = Z[i,:] otherwise.

Sharding: context axis (n+1) split 8 ways (1025 cols/core over a zero-padded
8200-wide array).  Each core computes r already broadcast across partitions
in one matmul per chunk (lamB[k,p] = lambda^k as the weight against the
shifted-window toeplitz), forms its partial s columns with multiply+reduce
against the bulk-loaded Ztop tiles, a 2KB DRAM AllGather + local sum
combines s across cores, then each core computes u for its columns.

Performance structure (final):
  - Bulk copy moves as bf16 (~1.7e-3 rel err, under the 2e-2 gate), halving
    DMA traffic.  The updated last row stays f32.
  - No transposed Ztop input: stage 2 is elementwise multiply+reduce against
    zbig rows 0..511, which are loaded first (tile-major order) with big
    descriptors.  This removes the 256-small-descriptor ztp load whose
    cross-core ring skew previously delayed the collective by ~30us.
  - Queues: SP carries the tiny loads, then the rows-0..511 half of the bulk
    load, then the last-row store; Act carries the rows-512..1023 half plus
    ALL bulk stores; the Pool/SWDGE queue carries ONLY the collective's
    DMAs -- pending SWDGE descriptors delay the CC mesh start, so keeping
    that queue empty is worth ~20us.
"""

import sys

for _p in ("/opt/trn_rl_repo", "/root/.axon_site/_ro/trn_rl_repo"):
    if _p not in sys.path:
        sys.path.append(_p)

import ml_dtypes
import numpy as np

import concourse.bacc as bacc
import concourse.bass as bass
import concourse.mybir as mybir
import concourse.tile as tile
from concourse.ap import AP
from concourse import bass_utils

F32 = mybir.dt.float32
BF16 = mybir.dt.bfloat16
NP_BF16 = ml_dtypes.bfloat16

D = 512          # feature dim d
N = 8192         # context length n
R = 2 * D + 1    # 1025 rows
NC = 8           # cores
L = 1025         # columns per core (8 * 1025 = 8200 >= 8193)
WTOT = NC * L    # 8200 padded width
W = 64           # geometric window taps
LPAD = 1152      # padded local column count (3 chunks of 384)
ZWLEN = 1280     # zwin input length: covers LPAD + W - 1
NT_K = D // 128      # 4 feature tiles
NT_ROW = 8           # full 128-row tiles (rows 0..1023)
RT_CHUNK = 384       # rT is produced in 3 chunks of 384 columns
J_CHUNKS = [(0, 512), (512, 1024), (1024, 1025)]

_PROGRAM = None


def _build_program():
    nc = bacc.Bacc(
        "TRN2",
        target_bir_lowering=False,
        debug=False,
        enable_asserts=False,
        num_devices=NC,
    )

    zc_d = nc.dram_tensor("zc", [128, NT_ROW, L], BF16, kind="ExternalInput")
    zla_d = nc.dram_tensor("zla", [L + 1], F32, kind="ExternalInput")
    zwin_d = nc.dram_tensor("zwin", [ZWLEN], BF16, kind="ExternalInput")
    lamb_d = nc.dram_tensor("lamb", [W, 128], BF16, kind="ExternalInput")
    out_d = nc.dram_tensor("out", [128, NT_ROW, L], BF16, kind="ExternalOutput")
    outlast_d = nc.dram_tensor("outlast", [L], F32, kind="ExternalOutput")

    with tile.TileContext(nc) as tc:
        with (
            tc.tile_pool(name="consts", bufs=1) as consts,
            tc.tile_pool(name="zbuf", bufs=1) as zbuf,
            tc.tile_pool(name="work", bufs=1) as work,
            tc.tile_pool(name="rt_ps", bufs=2, space=bass.MemorySpace.PSUM) as rt_ps,
            tc.tile_pool(name="rb_ps", bufs=2, space=bass.MemorySpace.PSUM) as rb_ps,
            tc.tile_pool(name="u_ps", bufs=2, space=bass.MemorySpace.PSUM) as u_ps,
            tc.tile_pool(name="ccdram", bufs=1, space="DRAM") as ccdram,
        ):
            # ---- SP queue: critical small loads + the 0..511-row half of
            # the bulk load (partitions 0-63) ------------------------------
            lamB = consts.tile([W, 128], BF16, name="lamB")
            nc.sync.dma_start(lamB[:], lamb_d[:, :])

            # overlapping window: win[k, j] = zwin[k + j]
            win = consts.tile([W, LPAD], BF16, name="win")
            nc.sync.dma_start(win[:], AP(zwin_d, 0, [[1, W], [1, LPAD]]))

            # one call for the whole A half: fewer DMA calls -> fewer CC
            # bookkeeping events ahead of the collective trigger
            zbig = zbuf.tile([128, NT_ROW, L], BF16, name="zbig")
            nc.sync.dma_start(zbig[:, 0:4, :], zc_d[:, 0:4, :])

            zla = work.tile([1, L + 1], F32, name="zla")
            nc.sync.dma_start(zla[:], zla_d[:].unsqueeze(0))
            zlast = zla[0:1, 0:L]

            scale_sb = consts.tile([1, 1], F32, name="scale_sb")
            nc.vector.tensor_scalar_mul(scale_sb[:], zla[0:1, L:L + 1],
                                        1.0 / float(N))

            # ---- stage 1: rbc = lamB.T @ win directly gives the r row
            # broadcast across all 128 partitions (lamB[k, p] = lambda^k) --
            rbc = work.tile([128, LPAD], BF16, name="rbc")
            for c in range(3):
                c0, c1 = c * RT_CHUNK, (c + 1) * RT_CHUNK
                rb = rb_ps.tile([128, RT_CHUNK], F32, name="rb", tag="rb")
                nc.tensor.matmul(rb[:], lamB[:], win[:, c0:c1],
                                 start=True, stop=True)
                nc.vector.tensor_copy(rbc[:, c0:c1], rb[:])

            # ---- stage 2: fused multiply+reduce against Ztop tiles ------
            # s[i] = sum_j zbig[i, j] * r[j] for the 4 feature tiles
            s_sb = work.tile([128, NT_K], F32, name="s_sb")
            for kt in range(NT_K):
                prod = work.tile([128, L], BF16, name=f"prod{kt}", tag=f"prod{kt}")
                nc.vector.tensor_mul(prod[:], zbig[:, kt, :], rbc[:, 0:L])
                nc.vector.tensor_reduce(
                    s_sb[:, kt:kt + 1], prod[:],
                    mybir.AxisListType.X, mybir.AluOpType.add,
                )

            # delay B's generation until the local partial s is done, so
            # only ~5 DMA-call events per core precede the mesh trigger
            gate_dr = ccdram.tile([1, NT_K], F32, name="gate_dr")
            nc.scalar.dma_start(gate_dr[:], s_sb[127:128, :])
            nc.scalar.dma_start(zbig[:, 4:8, :], zc_d[:, 4:8, :])

            # ---- AllGather partial s (2 KB) + local sum -----------------
            cc_in = ccdram.tile([128, NT_K], F32, name="cc_in")
            cc_out = ccdram.tile([NC * 128, NT_K], F32, name="cc_out")
            nc.gpsimd.dma_start(cc_in[:], s_sb[:])
            nc.gpsimd.collective_compute(
                "AllGather",
                mybir.AluOpType.bypass,
                replica_groups=[list(range(NC))],
                ins=[cc_in.opt()],
                outs=[cc_out.opt()],
            )
            sg = work.tile([128, NC, NT_K], F32, name="sg")
            nc.gpsimd.dma_start(sg[:], cc_out.rearrange("(r p) c -> p r c", p=128))

            # ---- bulk store: one call on the Act queue ------------------
            nc.scalar.dma_start(out_d[:, :, :], zbig[:, :, :])

            # ---- stage 3 prep: zd = Zmid - Ztop (emitted before the
            # post-collective vector work so the in-order vector engine
            # isn't stalled on the mesh) ----------------------------------
            zd = []
            for kt in range(NT_K):
                zd_t = work.tile([128, L], BF16, name=f"zd{kt}", tag=f"zd{kt}")
                nc.vector.tensor_sub(zd_t[:], zbig[:, NT_K + kt, :], zbig[:, kt, :])
                zd.append(zd_t)

            # ---- post-collective: sum the 8 partial s, cast to bf16 -----
            ssum = work.tile([128, NT_K], F32, name="ssum")
            nc.vector.tensor_add(ssum[:], sg[:, 0, :], sg[:, 1, :])
            for r_ in range(2, NC):
                nc.vector.tensor_add(ssum[:], ssum[:], sg[:, r_, :])
            ssum_bf = work.tile([128, NT_K], BF16, name="ssum_bf")
            nc.vector.tensor_copy(ssum_bf[:], ssum[:])

            # ---- stage 3: u = zd.T @ s; last row = zlast + scale*u ------
            newrow = work.tile([1, L], F32, name="newrow")
            for (j0, j1) in J_CHUNKS:
                u = u_ps.tile([1, j1 - j0], F32, name="u", tag="u")
                for kt in range(NT_K):
                    nc.tensor.matmul(
                        u[:], ssum_bf[:, kt:kt + 1], zd[kt][:, j0:j1],
                        start=(kt == 0), stop=(kt == NT_K - 1),
                    )
                nc.vector.scalar_tensor_tensor(
                    newrow[:, j0:j1], u[:], scale_sb[:], zla[0:1, j0:j1],
                    op0=mybir.AluOpType.mult, op1=mybir.AluOpType.add,
                )
            nc.sync.dma_start(outlast_d[:].unsqueeze(0), newrow[:])

    nc.compile()
    return nc


def _get_program():
    global _PROGRAM
    if _PROGRAM is None:
        _PROGRAM = _build_program()
    return _PROGRAM


def _make_in_maps(Z, alpha, M=None):
    Z = np.asarray(Z, dtype=np.float32)
    alpha = np.asarray(alpha, dtype=np.float32).reshape(1)
    # lambda powers; prefer deriving from M's first column when provided.
    if M is not None:
        lam = np.ascontiguousarray(np.asarray(M)[0:W, 0], dtype=np.float32)
    else:
        lam = (0.9 ** np.arange(W)).astype(np.float32)
    lamb_bf = np.ascontiguousarray(
        np.broadcast_to(lam[:, None], (W, 128))
    ).astype(NP_BF16)

    Zp = np.zeros((R, WTOT), dtype=np.float32)
    Zp[:, : N + 1] = Z
    zmpad = np.zeros(WTOT + ZWLEN, dtype=np.float32)
    zmpad[:N] = Z[R - 1, :N]  # col n masked to zero (M's last row is zero)

    in_maps = []
    for c in range(NC):
        j0 = c * L
        shard = Zp[:, j0:j0 + L]
        # rows 0..1023 permuted: zc[p, t, :] = shard[t*128 + p, :], bf16
        zc = np.ascontiguousarray(
            shard[:1024].reshape(NT_ROW, 128, L).transpose(1, 0, 2)
        ).astype(NP_BF16)
        in_maps.append(
            {
                "zc": zc,
                "zla": np.concatenate(
                    [np.ascontiguousarray(shard[R - 1]), alpha]
                ).astype(np.float32),
                "zwin": np.ascontiguousarray(zmpad[j0:j0 + ZWLEN]).astype(NP_BF16),
                "lamb": lamb_bf,
            }
        )
    return in_maps


def kernel(Z, alpha, P=None, M=None, Q=None, **_ignored):
    nc = _get_program()
    in_maps = _make_in_maps(Z, alpha, M)
    res = bass_utils.run_bass_kernel_spmd(nc, in_maps, core_ids=list(range(NC)))
    full = np.zeros((R, WTOT), dtype=np.float32)
    for c in range(NC):
        j0 = c * L
        rows = (
            res.results[c]["out"].astype(np.float32)
            .transpose(1, 0, 2).reshape(1024, L)
        )
        full[:1024, j0:j0 + L] = rows
        full[R - 1, j0:j0 + L] = res.results[c]["outlast"]
    return full[:, : N + 1].astype(np.float32)
